# revision 7
# baseline (speedup 1.0000x reference)
"""Trainium2 Bass kernel for nn_DA_conv: per-sample generated depthwise 3x3 conv
-> relu -> 1x1 pointwise conv (+bias) -> + x * channel_attention(altitude).

Data-parallel over batch: 8 samples -> 8 NeuronCores, weights replicated.

Per-core device pipeline:
  prologue:  feat = lrelu(W1 @ alt);  ktab[c,t] = <feat, W2[c*9+t,:]> (9 tiny matmuls)
             att = sigmoid(ca_w2 @ lrelu(ca_w1 @ alt))
             diag_t = diag(ktab[:,t]) built with one DVE tensor_scalar each
  main loop (chunks of 32 image rows, psum blocks of 4 rows = 512 f32):
    PE  : 9 diagonal matmuls (float32r) accumulate the depthwise conv in PSUM
    ACT : relu(psum_s) -> SBUF
    PE  : pointwise conv_w.T matmul + diag(att) @ x residual into psum_o
    DVE : osb = psum_o + conv_b  (evacuation with fused bias)
    DMA : osb -> DRAM

The host zero-pads x to [C, 130, 130] so every tap is a pure access-pattern
offset (no edge handling on device).
"""

import os
import numpy as np
from contextlib import ExitStack

import concourse.bass as bass
import concourse.mybir as mybir
import concourse.tile as tile
from concourse import bacc
from concourse.bass_utils import run_bass_kernel_spmd

AF = mybir.ActivationFunctionType
ALU = mybir.AluOpType
F32 = mybir.dt.float32
F32R = mybir.dt.float32r

B, C, H, W = 8, 128, 128, 128
KK = 3
NT = KK * KK                 # 9 taps
HW = H * W
WP = W + 2                   # host-padded width
HP = H + 2                   # host-padded height
R = 32                       # image rows per chunk
NCH = H // R                 # chunks
BR = 4                       # image rows per psum block (BR*W = 512 fp32 = 1 bank)
NBLK = R // BR               # psum blocks per chunk
TAPS = [(dy, dx) for dy in (-1, 0, 1) for dx in (-1, 0, 1)]  # t = (dy+1)*3+(dx+1)

last_results = None          # BassKernelResults of the most recent run


def _emit(tc, nc, d):
    """Emit the per-core program. d: dict of DRAM APs."""
    ctx = d["ctx"]
    singles = ctx.enter_context(tc.tile_pool(name="singles", bufs=1))
    xpool = ctx.enter_context(tc.tile_pool(name="xpool", bufs=3))
    spool = ctx.enter_context(tc.tile_pool(name="spool", bufs=2))
    opool = ctx.enter_context(tc.tile_pool(name="opool", bufs=2))
    pss_pool = ctx.enter_context(tc.tile_pool(name="psum_s", bufs=3, space="PSUM"))
    pso_pool = ctx.enter_context(tc.tile_pool(name="psum_o", bufs=3, space="PSUM"))
    pm_pool = ctx.enter_context(tc.tile_pool(name="psum_m", bufs=2, space="PSUM"))

    def load(name, dram, shape, dt=F32):
        t = singles.tile(shape, dt, name=name, tag=name)
        nc.sync.dma_start(out=t, in_=dram)
        return t

    alt = load("alt_s", d["alt"], [128, 1])
    w1t = load("w1t_s", d["w1t"], [128, 128])
    w2t = load("w2t_s", d["w2t"], [128, C * NT])
    cwt = load("cwt_s", d["cwt"], [C, C], dt=F32R)
    cb = load("cb_s", d["cb"], [C, 1])
    ca1t = load("ca1t_s", d["ca1t"], [128, 16])
    ca2t = load("ca2t_s", d["ca2t"], [16, 128])
    iota = load("iota_s", d["iota"], [128, 128])
    cidx = load("cidx_s", d["cidx"], [128, 1])

    def leaky(name, psum_src, parts):
        """lrelu(v) = max(0.1*v, v), via ACT copy to SBUF then one DVE STT."""
        tmp = singles.tile([parts, 1], F32, name=f"{name}_t", tag=f"{name}_t")
        nc.scalar.activation(tmp, psum_src, AF.Copy)
        res = singles.tile([parts, 1], F32, name=name, tag=name)
        nc.vector.scalar_tensor_tensor(
            out=res, in0=tmp, scalar=0.1, in1=tmp, op0=ALU.mult, op1=ALU.max
        )
        return res

    # ---- kernel-generator MLP ----
    feat_ps = pm_pool.tile([128, 1], F32, name="feat_ps", tag="pm")
    nc.tensor.matmul(feat_ps, lhsT=w1t, rhs=alt, start=True, stop=True)
    feat = leaky("feat", feat_ps, 128)

    ktab_ps = pm_pool.tile([128, NT], F32, name="ktab_ps", tag="pm")
    w2r = w2t.rearrange("p (c t) -> p t c", t=NT)
    for t in range(NT):
        nc.tensor.matmul(
            ktab_ps[:, t : t + 1], lhsT=w2r[:, t, :], rhs=feat, start=True, stop=True
        )
    ktab = singles.tile([128, NT], F32, name="ktab", tag="ktab")
    nc.scalar.activation(ktab, ktab_ps, AF.Copy)

    # ---- channel attention ----
    a1_ps = pm_pool.tile([16, 1], F32, name="a1_ps", tag="pm")
    nc.tensor.matmul(a1_ps, lhsT=ca1t, rhs=alt, start=True, stop=True)
    a1 = leaky("a1", a1_ps, 16)
    att_ps = pm_pool.tile([128, 1], F32, name="att_ps", tag="pm")
    nc.tensor.matmul(att_ps, lhsT=ca2t, rhs=a1, start=True, stop=True)
    attv = singles.tile([128, 1], F32, name="attv", tag="attv")
    nc.scalar.activation(attv, att_ps, AF.Sigmoid)

    # ---- diagonal weight matrices: diag(ktab[:, t]) and diag(att) ----
    diags = []
    for t in range(NT):
        dg = singles.tile([128, 128], F32R, name=f"diag{t}", tag=f"diag{t}")
        nc.vector.tensor_scalar(
            out=dg, in0=iota, scalar1=cidx, scalar2=ktab[:, t : t + 1],
            op0=ALU.is_equal, op1=ALU.mult,
        )
        diags.append(dg)
    attd = singles.tile([128, 128], F32R, name="attd", tag="attd")
    nc.vector.tensor_scalar(
        out=attd, in0=iota, scalar1=cidx, scalar2=attv,
        op0=ALU.is_equal, op1=ALU.mult,
    )

    x3 = d["xpad"].rearrange("c (h w) -> c h w", w=WP)
    out_d = d["out"]

    # ---- main loop, software-pipelined by one psum block ----
    pending_tail = None
    for ci in range(NCH):
        y0 = ci * R
        xp = xpool.tile([128, R + 2, WP], F32R, name=f"xp{ci}", tag="xp")
        nc.sync.dma_start(out=xp, in_=x3[:, y0 : y0 + R + 2, :])
        srelu = spool.tile([128, R * W], F32R, name=f"sr{ci}", tag="sr")
        osb = opool.tile([128, R * W], F32, name=f"ob{ci}", tag="ob")

        for b in range(NBLK):
            r0 = b * BR
            pss = pss_pool.tile([128, BR * W], F32, name=f"pss{ci}_{b}", tag="pss")
            for ti, (dy, dx) in enumerate(TAPS):
                rhs = xp[:, 1 + r0 + dy : 1 + r0 + dy + BR, 1 + dx : 1 + dx + W]
                nc.tensor.matmul(
                    pss, lhsT=diags[ti], rhs=rhs,
                    start=(ti == 0), stop=(ti == NT - 1),
                )
            if pending_tail is not None:
                pending_tail()
            pending_tail = _make_tail(nc, pso_pool, xp, srelu, osb, pss, cwt, attd,
                                      cb, out_d, ci, b, r0, y0)
        # keep the pipeline running across chunk boundaries
    if pending_tail is not None:
        pending_tail()


def _make_tail(nc, pso_pool, xp, srelu, osb, pss, cwt, attd, cb, out_d, ci, b, r0, y0):
    """relu + pointwise + residual + evac for block (ci, b); emitted one block late
    so the PE never waits on ACT's relu."""

    def tail():
        sl = slice(r0 * W, (r0 + BR) * W)
        nc.scalar.activation(srelu[:, sl], pss, AF.Relu)
        pso = pso_pool.tile([128, BR * W], F32, name=f"pso{ci}_{b}", tag="pso")
        nc.tensor.matmul(
            pso, lhsT=cwt, rhs=srelu[:, sl],
            start=True, stop=False,
        )
        nc.tensor.matmul(
            pso, lhsT=attd,
            rhs=xp[:, 1 + r0 : 1 + r0 + BR, 1 : 1 + W],
            start=False, stop=True,
        )
        nc.vector.tensor_scalar_add(out=osb[:, sl], in0=pso, scalar1=cb)
        if b == NBLK - 1:
            nc.sync.dma_start(out=out_d[:, y0 * W : (y0 + R) * W], in_=osb)

    return tail


def build_module():
    nc = bacc.Bacc(
        "TRN2",
        target_bir_lowering=False,
        debug=False,
        enable_asserts=False,
        num_devices=B,
    )
    d = {
        "xpad": nc.dram_tensor("xpad", [C, HP * WP], F32R, kind="ExternalInput").ap(),
        "alt": nc.dram_tensor("alt", [128, 1], F32, kind="ExternalInput").ap(),
        "w1t": nc.dram_tensor("w1t", [128, 128], F32, kind="ExternalInput").ap(),
        "w2t": nc.dram_tensor("w2t", [128, C * NT], F32, kind="ExternalInput").ap(),
        "cwt": nc.dram_tensor("cwt", [C, C], F32R, kind="ExternalInput").ap(),
        "cb": nc.dram_tensor("cb", [C, 1], F32, kind="ExternalInput").ap(),
        "ca1t": nc.dram_tensor("ca1t", [128, 16], F32, kind="ExternalInput").ap(),
        "ca2t": nc.dram_tensor("ca2t", [16, 128], F32, kind="ExternalInput").ap(),
        "iota": nc.dram_tensor("iota", [128, 128], F32, kind="ExternalInput").ap(),
        "cidx": nc.dram_tensor("cidx", [128, 1], F32, kind="ExternalInput").ap(),
        "out": nc.dram_tensor("out", [C, HW], F32, kind="ExternalOutput").ap(),
    }
    with tile.TileContext(nc) as tc:
        with ExitStack() as ctx:
            d["ctx"] = ctx
            _emit(tc, nc, d)
    nc.finalize()
    return nc


_module_cache = None


def _get_module():
    global _module_cache
    if _module_cache is None:
        _module_cache = build_module()
    return _module_cache


def make_in_maps(x, altitude, W1, W2, conv_w, conv_b, ca_w1, ca_w2):
    f = np.float32
    x = np.asarray(x, dtype=f)
    altitude = np.asarray(altitude, dtype=f)
    xpad = np.zeros((B, C, HP, WP), dtype=f)
    xpad[:, :, 1 : H + 1, 1 : W + 1] = x
    shared = {
        "w1t": np.ascontiguousarray(np.asarray(W1, dtype=f).T),
        "w2t": np.ascontiguousarray(np.asarray(W2, dtype=f).T),
        "cwt": np.ascontiguousarray(np.asarray(conv_w, dtype=f).T),
        "cb": np.ascontiguousarray(np.asarray(conv_b, dtype=f).reshape(C, 1)),
        "ca1t": np.ascontiguousarray(np.asarray(ca_w1, dtype=f).T),
        "ca2t": np.ascontiguousarray(np.asarray(ca_w2, dtype=f).T),
        "iota": np.ascontiguousarray(
            np.broadcast_to(np.arange(128, dtype=f), (128, 128))
        ),
        "cidx": np.arange(128, dtype=f).reshape(128, 1).copy(),
    }
    return [
        dict(
            shared,
            xpad=np.ascontiguousarray(xpad[bb].reshape(C, HP * WP)),
            alt=np.ascontiguousarray(altitude[bb].reshape(128, 1)),
        )
        for bb in range(B)
    ]


def kernel(x, altitude, W1, W2, conv_w, conv_b, ca_w1, ca_w2):
    global last_results
    in_maps = make_in_maps(x, altitude, W1, W2, conv_w, conv_b, ca_w1, ca_w2)
    nc = _get_module()
    trace = os.environ.get("KERNEL_TRACE", "0") == "1"
    last_results = run_bass_kernel_spmd(
        nc, in_maps, core_ids=list(range(B)), trace=trace
    )
    out = np.stack(
        [last_results.results[bb]["out"].reshape(C, H, W) for bb in range(B)]
    )
    return out


# revision 10
# speedup vs baseline: 1.0648x; 1.0648x over previous
"""Trainium2 Bass kernel for nn_DA_conv: per-sample generated depthwise 3x3 conv
-> relu -> 1x1 pointwise conv (+bias) -> + x * channel_attention(altitude).

Data-parallel over batch: 8 samples -> 8 NeuronCores, weights replicated.

Per-core device pipeline:
  prologue:  feat = lrelu(W1 @ alt);  ktab[c,t] = <feat, W2[c*9+t,:]> (9 tiny matmuls)
             att = sigmoid(ca_w2 @ lrelu(ca_w1 @ alt))
             diag_t = diag(ktab[:,t]) built with one DVE tensor_scalar each
  main loop (chunks of 32 image rows, psum blocks of 4 rows = 512 f32):
    PE  : 9 diagonal matmuls (float32r) accumulate the depthwise conv in PSUM
    ACT : relu(psum_s) -> SBUF
    PE  : pointwise conv_w.T matmul + diag(att) @ x residual into psum_o
    DVE : osb = psum_o + conv_b  (evacuation with fused bias)
    DMA : osb -> DRAM

The host zero-pads x to [C, 130, 130] so every tap is a pure access-pattern
offset (no edge handling on device).
"""

import os
import ml_dtypes
import numpy as np
from contextlib import ExitStack

import concourse.bass as bass
import concourse.mybir as mybir
import concourse.tile as tile
from concourse import bacc
from concourse.bass_utils import run_bass_kernel_spmd

AF = mybir.ActivationFunctionType
ALU = mybir.AluOpType
F32 = mybir.dt.float32
F32R = mybir.dt.float32r
BF16 = mybir.dt.bfloat16

B, C, H, W = 8, 128, 128, 128
KK = 3
NT = KK * KK                 # 9 taps
HW = H * W
WP = W + 2                   # host-padded width
HP = H + 2                   # host-padded height
R = 32                       # image rows per chunk
NCH = H // R                 # chunks
BR = 4                       # image rows per psum block (BR*W = 512 fp32 = 1 bank)
NBLK = R // BR               # psum blocks per chunk
TAPS = [(dy, dx) for dy in (-1, 0, 1) for dx in (-1, 0, 1)]  # t = (dy+1)*3+(dx+1)

last_results = None          # BassKernelResults of the most recent run


def _emit(tc, nc, d):
    """Emit the per-core program. d: dict of DRAM APs."""
    ctx = d["ctx"]
    singles = ctx.enter_context(tc.tile_pool(name="singles", bufs=1))
    xpool = ctx.enter_context(tc.tile_pool(name="xpool", bufs=3))
    spool = ctx.enter_context(tc.tile_pool(name="spool", bufs=2))
    opool = ctx.enter_context(tc.tile_pool(name="opool", bufs=2))
    pss_pool = ctx.enter_context(tc.tile_pool(name="psum_s", bufs=3, space="PSUM"))
    pso_pool = ctx.enter_context(tc.tile_pool(name="psum_o", bufs=3, space="PSUM"))
    pm_pool = ctx.enter_context(tc.tile_pool(name="psum_m", bufs=2, space="PSUM"))

    def load(name, dram, shape, dt=F32):
        t = singles.tile(shape, dt, name=name, tag=name)
        nc.sync.dma_start(out=t, in_=dram)
        return t

    alt = load("alt_s", d["alt"], [128, 1])
    w1t = load("w1t_s", d["w1t"], [128, 128])
    w2t = load("w2t_s", d["w2t"], [128, C * NT])
    cwt = load("cwt_s", d["cwt"], [C, C], dt=BF16)
    cb = load("cb_s", d["cb"], [C, 1])
    ca1t = load("ca1t_s", d["ca1t"], [128, 16])
    ca2t = load("ca2t_s", d["ca2t"], [16, 128])
    iota = load("iota_s", d["iota"], [128, 128])
    cidx = load("cidx_s", d["cidx"], [128, 1])

    def leaky(name, psum_src, parts):
        """lrelu(v) = max(0.1*v, v), via ACT copy to SBUF then one DVE STT."""
        tmp = singles.tile([parts, 1], F32, name=f"{name}_t", tag=f"{name}_t")
        nc.scalar.activation(tmp, psum_src, AF.Copy)
        res = singles.tile([parts, 1], F32, name=name, tag=name)
        nc.vector.scalar_tensor_tensor(
            out=res, in0=tmp, scalar=0.1, in1=tmp, op0=ALU.mult, op1=ALU.max
        )
        return res

    # ---- kernel-generator MLP ----
    feat_ps = pm_pool.tile([128, 1], F32, name="feat_ps", tag="pm")
    nc.tensor.matmul(feat_ps, lhsT=w1t, rhs=alt, start=True, stop=True)
    feat = leaky("feat", feat_ps, 128)

    ktab_ps = pm_pool.tile([128, NT], F32, name="ktab_ps", tag="pm")
    w2r = w2t.rearrange("p (c t) -> p t c", t=NT)
    for t in range(NT):
        nc.tensor.matmul(
            ktab_ps[:, t : t + 1], lhsT=w2r[:, t, :], rhs=feat, start=True, stop=True
        )
    ktab = singles.tile([128, NT], F32, name="ktab", tag="ktab")
    nc.scalar.activation(ktab, ktab_ps, AF.Copy)

    # ---- channel attention ----
    a1_ps = pm_pool.tile([16, 1], F32, name="a1_ps", tag="pm")
    nc.tensor.matmul(a1_ps, lhsT=ca1t, rhs=alt, start=True, stop=True)
    a1 = leaky("a1", a1_ps, 16)
    att_ps = pm_pool.tile([128, 1], F32, name="att_ps", tag="pm")
    nc.tensor.matmul(att_ps, lhsT=ca2t, rhs=a1, start=True, stop=True)
    attv = singles.tile([128, 1], F32, name="attv", tag="attv")
    nc.scalar.activation(attv, att_ps, AF.Sigmoid)

    # ---- diagonal weight matrices: diag(ktab[:, t]) and diag(att) ----
    diags = []
    for t in range(NT):
        dg = singles.tile([128, 128], BF16, name=f"diag{t}", tag=f"diag{t}")
        nc.vector.tensor_scalar(
            out=dg, in0=iota, scalar1=cidx, scalar2=ktab[:, t : t + 1],
            op0=ALU.is_equal, op1=ALU.mult,
        )
        diags.append(dg)
    attd = singles.tile([128, 128], BF16, name="attd", tag="attd")
    nc.vector.tensor_scalar(
        out=attd, in0=iota, scalar1=cidx, scalar2=attv,
        op0=ALU.is_equal, op1=ALU.mult,
    )

    x3h = d["xpad_hi"].rearrange("c (h w) -> c h w", w=WP)
    x3l = d["xpad_lo"].rearrange("c (h w) -> c h w", w=WP)
    out_d = d["out"]

    # ---- main loop, software-pipelined by one psum block ----
    pending_tail = None
    for ci in range(NCH):
        y0 = ci * R
        xp = xpool.tile([128, R + 2, WP], BF16, name=f"xp{ci}", tag="xp")
        nc.sync.dma_start(out=xp, in_=x3h[:, y0 : y0 + R + 2, :])
        xpl = xpool.tile([128, R + 2, WP], BF16, name=f"xpl{ci}", tag="xpl")
        nc.sync.dma_start(out=xpl, in_=x3l[:, y0 : y0 + R + 2, :])
        srelu = spool.tile([128, R * W], BF16, name=f"sr{ci}", tag="sr")
        osb = opool.tile([128, R * W], F32, name=f"ob{ci}", tag="ob")

        for b in range(NBLK):
            r0 = b * BR
            pss = pss_pool.tile([128, BR * W], F32, name=f"pss{ci}_{b}", tag="pss")
            for ti, (dy, dx) in enumerate(TAPS):
                rhs = xp[:, 1 + r0 + dy : 1 + r0 + dy + BR, 1 + dx : 1 + dx + W]
                nc.tensor.matmul(
                    pss, lhsT=diags[ti], rhs=rhs,
                    start=(ti == 0), stop=(ti == NT - 1),
                )
            if pending_tail is not None:
                pending_tail()
            pending_tail = _make_tail(nc, pso_pool, xp, xpl, srelu, osb, pss, cwt,
                                      attd, attv, cb, out_d, ci, b, r0, y0)
        # keep the pipeline running across chunk boundaries
    if pending_tail is not None:
        pending_tail()


def _make_tail(nc, pso_pool, xp, xpl, srelu, osb, pss, cwt, attd, attv, cb, out_d,
               ci, b, r0, y0):
    """relu + pointwise + residual + evac for block (ci, b); emitted one block late
    so the PE never waits on ACT's relu.

    residual att*x = att_f32*x_hi (DVE STT during evac) + diag(att_bf16)@x_lo (PE);
    the bf16 rounding of att only touches the tiny x_lo term."""

    def tail():
        sl = slice(r0 * W, (r0 + BR) * W)
        nc.scalar.activation(srelu[:, sl], pss, AF.Relu)
        pso = pso_pool.tile([128, BR * W], F32, name=f"pso{ci}_{b}", tag="pso")
        nc.tensor.matmul(
            pso, lhsT=cwt, rhs=srelu[:, sl],
            start=True, stop=False,
        )
        nc.tensor.matmul(
            pso, lhsT=attd,
            rhs=xpl[:, 1 + r0 : 1 + r0 + BR, 1 : 1 + W],
            start=False, stop=True,
        )
        nc.vector.scalar_tensor_tensor(
            out=osb[:, sl], in0=xp[:, 1 + r0 : 1 + r0 + BR, 1 : 1 + W],
            scalar=attv, in1=pso, op0=ALU.mult, op1=ALU.add,
        )
        nc.vector.tensor_scalar_add(out=osb[:, sl], in0=osb[:, sl], scalar1=cb)
        if b == NBLK - 1:
            nc.sync.dma_start(out=out_d[:, y0 * W : (y0 + R) * W], in_=osb)

    return tail


def build_module():
    nc = bacc.Bacc(
        "TRN2",
        target_bir_lowering=False,
        debug=False,
        enable_asserts=False,
        num_devices=B,
    )
    d = {
        "xpad_hi": nc.dram_tensor("xpad_hi", [C, HP * WP], BF16, kind="ExternalInput").ap(),
        "xpad_lo": nc.dram_tensor("xpad_lo", [C, HP * WP], BF16, kind="ExternalInput").ap(),
        "alt": nc.dram_tensor("alt", [128, 1], F32, kind="ExternalInput").ap(),
        "w1t": nc.dram_tensor("w1t", [128, 128], F32, kind="ExternalInput").ap(),
        "w2t": nc.dram_tensor("w2t", [128, C * NT], F32, kind="ExternalInput").ap(),
        "cwt": nc.dram_tensor("cwt", [C, C], BF16, kind="ExternalInput").ap(),
        "cb": nc.dram_tensor("cb", [C, 1], F32, kind="ExternalInput").ap(),
        "ca1t": nc.dram_tensor("ca1t", [128, 16], F32, kind="ExternalInput").ap(),
        "ca2t": nc.dram_tensor("ca2t", [16, 128], F32, kind="ExternalInput").ap(),
        "iota": nc.dram_tensor("iota", [128, 128], F32, kind="ExternalInput").ap(),
        "cidx": nc.dram_tensor("cidx", [128, 1], F32, kind="ExternalInput").ap(),
        "out": nc.dram_tensor("out", [C, HW], F32, kind="ExternalOutput").ap(),
    }
    with tile.TileContext(nc) as tc:
        with ExitStack() as ctx:
            d["ctx"] = ctx
            _emit(tc, nc, d)
    nc.finalize()
    return nc


_module_cache = None


def _get_module():
    global _module_cache
    if _module_cache is None:
        _module_cache = build_module()
    return _module_cache


def make_in_maps(x, altitude, W1, W2, conv_w, conv_b, ca_w1, ca_w2):
    f = np.float32
    x = np.asarray(x, dtype=f)
    altitude = np.asarray(altitude, dtype=f)
    xpad = np.zeros((B, C, HP, WP), dtype=f)
    xpad[:, :, 1 : H + 1, 1 : W + 1] = x
    xhi_f = xpad.astype(ml_dtypes.bfloat16)
    xlo = np.ascontiguousarray(
        (xpad - xhi_f.astype(f)).astype(ml_dtypes.bfloat16).reshape(B, C, HP * WP)
    )
    xhi = np.ascontiguousarray(xhi_f.reshape(B, C, HP * WP))
    shared = {
        "w1t": np.ascontiguousarray(np.asarray(W1, dtype=f).T),
        "w2t": np.ascontiguousarray(np.asarray(W2, dtype=f).T),
        "cwt": np.ascontiguousarray(np.asarray(conv_w, dtype=f).T.astype(ml_dtypes.bfloat16)),
        "cb": np.ascontiguousarray(np.asarray(conv_b, dtype=f).reshape(C, 1)),
        "ca1t": np.ascontiguousarray(np.asarray(ca_w1, dtype=f).T),
        "ca2t": np.ascontiguousarray(np.asarray(ca_w2, dtype=f).T),
        "iota": np.ascontiguousarray(
            np.broadcast_to(np.arange(128, dtype=f), (128, 128))
        ),
        "cidx": np.arange(128, dtype=f).reshape(128, 1).copy(),
    }
    return [
        dict(
            shared,
            xpad_hi=xhi[bb],
            xpad_lo=xlo[bb],
            alt=np.ascontiguousarray(altitude[bb].reshape(128, 1)),
        )
        for bb in range(B)
    ]


def kernel(x, altitude, W1, W2, conv_w, conv_b, ca_w1, ca_w2):
    global last_results
    in_maps = make_in_maps(x, altitude, W1, W2, conv_w, conv_b, ca_w1, ca_w2)
    nc = _get_module()
    trace = os.environ.get("KERNEL_TRACE", "0") == "1"
    last_results = run_bass_kernel_spmd(
        nc, in_maps, core_ids=list(range(B)), trace=trace
    )
    out = np.stack(
        [last_results.results[bb]["out"].reshape(C, H, W) for bb in range(B)]
    )
    return out
